# revision 4
# baseline (speedup 1.0000x reference)
"""Trainium2 Bass kernel v3 for nn_MultiHeadAttention (GQA + RoPE + causal).

Same sharding/layout as v2 (4 batches x 2 head-halves, bf16 operands,
SBUF-resident x/Q/K/V/Y, multiplicative causal mask, DVE-tree softmax
denominator). v3 schedule/engine-balance changes:
  - x DMA split into 4 strips so K-proj starts after the first strip lands
  - wq resident (loaded once, early) instead of per-head strips
  - rope elementwise math in bf16 on DVE (2x/4x modes); trig tables bf16
  - last diagonal pair of each (head, block): S/exp computed only on the
    visible query region, masked region of P zeroed by Pool memset
  - pbs (softmax broadcast) copy moved to DVE; O-proj psum->sbuf copies
    alternate ACT/DVE to kill the per-oc stall
"""

import sys
import math

sys.path.insert(0, "/opt/trn_rl_repo")

import numpy as np
import ml_dtypes

import concourse.bacc as bacc
import concourse.mybir as mybir
import concourse.tile as tile
from concourse.bass_utils import run_bass_kernel_spmd

F32 = mybir.dt.float32
F32R = mybir.dt.float32r
BF16 = mybir.dt.bfloat16
AF = mybir.ActivationFunctionType

B, T, C = 4, 2048, 2048
NH, NKV, HD = 16, 4, 128
NHL = NH // 2
NKVL = NKV // 2
NREP = NH // NKV
ROPE_BASE = 10000.0
NCC = C // 128
NJ = T // 512
NTC = T // 128
BF = ml_dtypes.bfloat16


PHASES = "all"


def _build_nc(nrep=1, extra_dma=0):
    nc = bacc.Bacc(trn_type="TRN2", name="mha_gqa_rope_v3")

    xT = nc.dram_tensor("xT", [C, T], BF16, kind="ExternalInput")
    wqT = nc.dram_tensor("wqT", [C, NHL * HD], BF16, kind="ExternalInput")
    wkT = nc.dram_tensor("wkT", [C, NKVL * HD], BF16, kind="ExternalInput")
    wvT = nc.dram_tensor("wvT", [C, NKVL * HD], BF16, kind="ExternalInput")
    woT = nc.dram_tensor("woT", [NHL * HD, C], BF16, kind="ExternalInput")
    cosk = nc.dram_tensor("cosk", [HD, T], BF16, kind="ExternalInput")
    sink = nc.dram_tensor("sink", [HD, T], BF16, kind="ExternalInput")
    maskmul = nc.dram_tensor("maskmul", [128, 4 * 512], BF16, kind="ExternalInput")
    ones_d = nc.dram_tensor("ones_d", [128, 128], F32R, kind="ExternalInput")
    onesb_d = nc.dram_tensor("onesb_d", [128, 1], BF16, kind="ExternalInput")
    outT = nc.dram_tensor("outT", [C, T], BF16, kind="ExternalOutput")

    with tile.TileContext(nc) as tc:
        with tc.tile_pool(name="const", bufs=1) as constp:
            ones_s = constp.tile([128, 128], F32R)
            nc.sync.dma_start(out=ones_s[:], in_=ones_d.ap())
            onesb_s = constp.tile([128, 1], BF16)
            nc.sync.dma_start(out=onesb_s[:], in_=onesb_d.ap())
            mask_s = constp.tile([128, 4, 512], BF16)
            nc.sync.dma_start(
                out=mask_s[:], in_=maskmul.ap().rearrange("p (i q) -> p i q", i=4)
            )

            for _rep in range(nrep):
                with tc.tile_pool(name="kvres", bufs=1) as kvres, \
                     tc.tile_pool(name="qy", bufs=1) as qyp:
                    kT_s = kvres.tile([128, NKVL, T], BF16)
                    v_s = kvres.tile([128, NTC, NKVL * HD], BF16)
                    q_s = qyp.tile([128, NHL, T], BF16)
                    y_s = qyp.tile([128, NHL, T], BF16)

                    with tc.tile_pool(name="xtrig", bufs=1) as xtp:
                        # small weights first so K-proj can start immediately
                        wk_s = xtp.tile([128, NCC, NKVL * HD], BF16, tag="wk")
                        nc.sync.dma_start(
                            out=wk_s[:],
                            in_=wkT.ap().rearrange("(c p) k -> p c k", p=128),
                        )
                        x_s = xtp.tile([128, NCC, T], BF16, tag="x")

                        def _load_x_strip(c0, c1):
                            nc.sync.dma_start(
                                out=x_s[:, c0:c1, :],
                                in_=xT.ap()[c0 * 128:c1 * 128, :].rearrange(
                                    "(c p) t -> p c t", p=128
                                ),
                            )

                        _load_x_strip(0, 4)
                        wv_s = xtp.tile([128, NCC, NKVL * HD], BF16, tag="wv")
                        nc.sync.dma_start(
                            out=wv_s[:],
                            in_=wvT.ap().rearrange("(c p) k -> p c k", p=128),
                        )
                        for s in range(1, 4):
                            _load_x_strip(4 * s, 4 * (s + 1))
                        cos_s = xtp.tile([HD, T], BF16, tag="cos")
                        nc.sync.dma_start(out=cos_s[:], in_=cosk.ap())
                        sin_s = xtp.tile([HD, T], BF16, tag="sin")
                        nc.sync.dma_start(out=sin_s[:], in_=sink.ap())
                        # wq strips streamed with prefetch (2 heads ahead)
                        wq_strips = []

                        def _issue_wq(wqp, h):
                            t = wqp.tile([128, NCC, 128], BF16, tag="wq")
                            nc.sync.dma_start(
                                out=t[:],
                                in_=wqT.ap()[:, h * 128:(h + 1) * 128].rearrange(
                                    "(c p) m -> p c m", p=128
                                ),
                            )
                            wq_strips.append(t)

                        # ---------------- KV stage ----------------
                        with tc.tile_pool(name="wqp", bufs=3) as wqp:
                          with tc.tile_pool(name="krope", bufs=2) as krp, \
                             tc.tile_pool(name="kps", bufs=2, space="PSUM") as kps, \
                             tc.tile_pool(name="vps", bufs=2, space="PSUM") as vps:
                            _issue_wq(wqp, 0)
                            _issue_wq(wqp, 1)
                            for g in range(NKVL):
                                for half in range(2):
                                    hsl = slice(half * 1024, (half + 1) * 1024)
                                    psk = kps.tile([128, 1024], F32, tag="psk")
                                    for cc in range(NCC):
                                        for rb in range(2):
                                            nc.tensor.matmul(
                                                psk[:, rb * 512:(rb + 1) * 512],
                                                wk_s[:, cc, g * 128:(g + 1) * 128],
                                                x_s[:, cc,
                                                    half * 1024 + rb * 512:
                                                    half * 1024 + (rb + 1) * 512],
                                                start=(cc == 0),
                                                stop=(cc == NCC - 1),
                                            )
                                    k0 = krp.tile([128, 1024], BF16, tag="k0")
                                    nc.scalar.copy(k0[:], psk[:])
                                    rot = krp.tile([128, 1024], BF16, tag="rot")
                                    nc.scalar.copy(rot[0:64, :], k0[64:128, :])
                                    nc.scalar.copy(rot[64:128, :], k0[0:64, :])
                                    t1 = krp.tile([128, 1024], BF16, tag="t1")
                                    nc.vector.tensor_mul(t1[:], k0[:], cos_s[:, hsl])
                                    nc.vector.tensor_mul(rot[:], rot[:], sin_s[:, hsl])
                                    nc.vector.tensor_add(kT_s[:, g, hsl], t1[:], rot[:])
                            for tc_i in range(NTC):
                                psv = vps.tile([128, NKVL * HD], F32, tag="psv")
                                for cc in range(NCC):
                                    nc.tensor.matmul(
                                        psv[:],
                                        x_s[:, cc, tc_i * 128:(tc_i + 1) * 128],
                                        wv_s[:, cc, :],
                                        start=(cc == 0),
                                        stop=(cc == NCC - 1),
                                    )
                                nc.scalar.copy(v_s[:, tc_i, :], psv[:])

                          # ------------- Q stage -------------
                          with tc.tile_pool(name="qrope", bufs=2) as qrp, \
                               tc.tile_pool(name="qps", bufs=2, space="PSUM") as qps:
                              for h in range(NHL):
                                if h + 2 < NHL:
                                    _issue_wq(wqp, h + 2)
                                wq_strip = wq_strips[h]
                                for half in range(2):
                                    hsl = slice(half * 1024, (half + 1) * 1024)
                                    psq = qps.tile([128, 1024], F32, tag="psq")
                                    for cc in range(NCC):
                                        for rb in range(2):
                                            nc.tensor.matmul(
                                                psq[:, rb * 512:(rb + 1) * 512],
                                                wq_strip[:, cc, :],
                                                x_s[:, cc,
                                                    half * 1024 + rb * 512:
                                                    half * 1024 + (rb + 1) * 512],
                                                start=(cc == 0),
                                                stop=(cc == NCC - 1),
                                            )
                                    q0 = qrp.tile([128, 1024], BF16, tag="q0")
                                    nc.scalar.copy(q0[:], psq[:])
                                    rot = qrp.tile([128, 1024], BF16, tag="qrot")
                                    nc.scalar.copy(rot[0:64, :], q0[64:128, :])
                                    nc.scalar.copy(rot[64:128, :], q0[0:64, :])
                                    t1 = qrp.tile([128, 1024], BF16, tag="qt1")
                                    nc.vector.tensor_mul(t1[:], q0[:], cos_s[:, hsl])
                                    nc.vector.tensor_mul(rot[:], rot[:], sin_s[:, hsl])
                                    nc.vector.tensor_add(q_s[:, h, hsl], t1[:], rot[:])

                    if PHASES == "proj":
                        # keep outputs live: copy a slice of q_s/kT_s out
                        with tc.tile_pool(name="dbg", bufs=1) as dbgp:
                            db = dbgp.tile([128, 512], BF16)
                            nc.vector.tensor_add(db[:], q_s[:, 0, 0:512],
                                                 kT_s[:, 0, 0:512])
                            nc.sync.dma_start(out=outT.ap()[0:128, 0:512],
                                              in_=db[:])
                        continue
                    # ---------------- attention + O-proj ----------------
                    with tc.tile_pool(name="wo", bufs=1) as wop, \
                         tc.tile_pool(name="pt", bufs=4) as ptp, \
                         tc.tile_pool(name="tree", bufs=8) as trp, \
                         tc.tile_pool(name="small", bufs=2) as smallp, \
                         tc.tile_pool(name="ob", bufs=4) as obp, \
                         tc.tile_pool(name="sps", bufs=2, space="PSUM") as sps, \
                         tc.tile_pool(name="po", bufs=2, space="PSUM") as ops, \
                         tc.tile_pool(name="den", bufs=1, space="PSUM") as dps, \
                         tc.tile_pool(name="pb", bufs=1, space="PSUM") as bps:
                        wo_s = wop.tile([128, NHL, C], BF16)
                        nc.sync.dma_start(
                            out=wo_s[:],
                            in_=woT.ap().rearrange("(yc p) o -> p yc o", p=128),
                        )
                        if extra_dma:
                            with tc.tile_pool(name="xd", bufs=2) as xdp:
                                for e in range(extra_dma * 4):
                                    xd = xdp.tile([128, 4, T], BF16, tag="xd")
                                    nc.sync.dma_start(
                                        out=xd[:],
                                        in_=xT.ap()[(e % 4) * 512:(e % 4 + 1) * 512, :]
                                        .rearrange("(c p) t -> p c t", p=128),
                                    )
                        self_cnt = [0]

                        def attend(j, h):
                                qsl = slice(j * 512, (j + 1) * 512)
                                g = h // NREP
                                po = ops.tile([128, 512], F32, tag="po")
                                npairs = 2 * j + 2
                                tree = []
                                for pp in range(npairs):
                                    last = pp == npairs - 1
                                    pss = sps.tile([128, 2, 512], F32, tag="pss")
                                    pt = ptp.tile([128, 2, 512], BF16, tag="pt")
                                    if not last:
                                        for ci in range(2):
                                            cc = 2 * pp + ci
                                            nc.tensor.matmul(
                                                pss[:, ci, :],
                                                kT_s[:, g, cc * 128:(cc + 1) * 128],
                                                q_s[:, h, qsl],
                                                start=True,
                                                stop=True,
                                            )
                                        nc.scalar.activation(
                                            pt[:].rearrange("p a q -> p (a q)"),
                                            pss[:].rearrange("p a q -> p (a q)"),
                                            AF.Exp,
                                        )
                                        for ci in range(2):
                                            cc = 2 * pp + ci
                                            if cc == 4 * j:
                                                nc.vector.tensor_mul(
                                                    pt[:, ci, 0:128],
                                                    pt[:, ci, 0:128],
                                                    mask_s[:, 0, 0:128],
                                                )
                                            elif cc == 4 * j + 1:
                                                nc.gpsimd.tensor_mul(
                                                    pt[:, ci, 0:256],
                                                    pt[:, ci, 0:256],
                                                    mask_s[:, 1, 0:256],
                                                )
                                    else:
                                        # diagonal pair: chunks 4j+2 (vis q>=256)
                                        # and 4j+3 (vis q>=384)
                                        for ci, vis in ((0, 256), (1, 384)):
                                            cc = 2 * pp + ci
                                            nc.tensor.matmul(
                                                pss[:, ci, vis:512],
                                                kT_s[:, g, cc * 128:(cc + 1) * 128],
                                                q_s[:, h,
                                                    j * 512 + vis:(j + 1) * 512],
                                                start=True,
                                                stop=True,
                                            )
                                            nc.gpsimd.memset(pt[:, ci, 0:vis], 0.0)
                                            nc.scalar.activation(
                                                pt[:, ci, vis:512],
                                                pss[:, ci, vis:512],
                                                AF.Exp,
                                            )
                                            if ci == 0:
                                                nc.vector.tensor_mul(
                                                    pt[:, ci, 256:384],
                                                    pt[:, ci, 256:384],
                                                    mask_s[:, 2, 256:384],
                                                )
                                            else:
                                                nc.gpsimd.tensor_mul(
                                                    pt[:, ci, 384:512],
                                                    pt[:, ci, 384:512],
                                                    mask_s[:, 3, 384:512],
                                                )
                                    for ci in range(2):
                                        cc = 2 * pp + ci
                                        nc.tensor.matmul(
                                            po[:],
                                            v_s[:, cc, g * 128:(g + 1) * 128],
                                            pt[:, ci, :],
                                            start=(cc == 0),
                                            stop=(cc == 4 * j + 3),
                                        )
                                    def tree_add(out_, a_, b_):
                                        eng = nc.vector
                                        self_cnt[0] += 1
                                        eng.tensor_add(out_, a_, b_)

                                    node = trp.tile([128, 512], BF16, tag="tn")
                                    tree_add(node[:], pt[:, 0, :], pt[:, 1, :])
                                    lvl = 0
                                    while tree and tree[-1][0] == lvl:
                                        prev = tree.pop()[1]
                                        nxt = trp.tile([128, 512], BF16, tag="tn")
                                        tree_add(nxt[:], prev[:], node[:])
                                        node = nxt
                                        lvl += 1
                                    tree.append((lvl, node))
                                while len(tree) > 1:
                                    a = tree.pop()[1]
                                    b_ = tree.pop()[1]
                                    nxt = trp.tile([128, 512], BF16, tag="tn")
                                    nc.vector.tensor_add(nxt[:], a[:], b_[:])
                                    tree.append((99, nxt))
                                root = tree[0][1]
                                den = dps.tile([1, 512], F32, tag="den")
                                nc.tensor.matmul(
                                    den[:], onesb_s[:, 0:1], root[:],
                                    start=True, stop=True,
                                )
                                rec = smallp.tile([1, 512], F32R, tag="rec")
                                with nc.allow_low_precision(reason="softmax recip"):
                                    nc.vector.reciprocal(rec[:], den[:])
                                pb = bps.tile([128, 512], F32, tag="pb")
                                nc.tensor.matmul(
                                    pb[:], ones_s[0:1, :], rec[:],
                                    start=True, stop=True,
                                )
                                pbs = smallp.tile([128, 512], F32, tag="pbs")
                                if h % 2 == 0:
                                    nc.scalar.copy(pbs[:], pb[:])
                                else:
                                    nc.vector.tensor_copy(pbs[:], pb[:])
                                nc.vector.tensor_mul(
                                    y_s[:, h, qsl], po[:], pbs[:]
                                )
                        def oproj(j):
                            qsl = slice(j * 512, (j + 1) * 512)
                            for oc in range(NCC):
                                pso = ops.tile([128, 512], F32, tag="po")
                                for yc in range(NHL):
                                    nc.tensor.matmul(
                                        pso[:],
                                        wo_s[:, yc, oc * 128:(oc + 1) * 128],
                                        y_s[:, yc, qsl],
                                        start=(yc == 0),
                                        stop=(yc == NHL - 1),
                                    )
                                ob = obp.tile([128, 512], BF16, tag="ob")
                                if oc % 2 == 0:
                                    nc.scalar.copy(ob[:], pso[:])
                                else:
                                    nc.vector.tensor_copy(ob[:], pso[:])
                                nc.sync.dma_start(
                                    out=outT.ap()[oc * 128:(oc + 1) * 128, qsl],
                                    in_=ob[:],
                                )

                        for ja, jb in ((0, 1), (2, 3)):
                            for h in range(NHL):
                                attend(ja, h)
                                attend(jb, h)
                            oproj(ja)
                            oproj(jb)

    nc.finalize()
    return nc


_NC_CACHE = None


def get_nc():
    global _NC_CACHE
    if _NC_CACHE is None:
        _NC_CACHE = _build_nc()
    return _NC_CACHE


def build_nrep(nrep):
    return _build_nc(nrep=nrep, extra_dma=EXTRA_DMA)


EXTRA_DMA = 0


def _trig_tables(offset):
    inv_freq = 1.0 / (ROPE_BASE ** (np.arange(0, HD, 2, dtype=np.float64) / HD))
    pos = np.arange(offset, offset + T, dtype=np.float64)
    ang = pos[:, None] * inv_freq[None, :]
    cos = np.cos(ang)
    sin = np.sin(ang)
    cosT = np.concatenate([cos, cos], axis=1).T.astype(np.float32)
    sinT = np.concatenate([-sin, sin], axis=1).T.astype(np.float32)
    return (np.ascontiguousarray(cosT).astype(BF),
            np.ascontiguousarray(sinT).astype(BF))


def _mask_table():
    k = np.arange(128)[:, None]
    q = np.arange(512)[None, :]
    blocks = [(128 * i + k <= q).astype(np.float32) for i in range(4)]
    return np.concatenate(blocks, axis=1).astype(BF)


def make_in_maps(x, Wq, Wk, Wv, Wo, offset):
    x = np.asarray(x, dtype=np.float32)
    Wq = np.asarray(Wq, dtype=np.float32)
    Wk = np.asarray(Wk, dtype=np.float32)
    Wv = np.asarray(Wv, dtype=np.float32)
    Wo = np.asarray(Wo, dtype=np.float32)
    offset = int(np.asarray(offset))

    scale = 1.0 / math.sqrt(HD)
    cosT, sinT = _trig_tables(offset)
    mask = _mask_table()
    ones = np.ones((128, 128), dtype=np.float32)
    onesb = np.ones((128, 1), dtype=np.float32).astype(BF)

    xTb = [np.ascontiguousarray(x[b].T).astype(BF) for b in range(B)]
    wq_h, wk_h, wv_h, wo_h = [], [], [], []
    for hh in range(2):
        qrows = slice(hh * NHL * HD, (hh + 1) * NHL * HD)
        kvrows = slice(hh * NKVL * HD, (hh + 1) * NKVL * HD)
        wq_h.append(np.ascontiguousarray((Wq[qrows] * scale).T).astype(BF))
        wk_h.append(np.ascontiguousarray(Wk[kvrows].T).astype(BF))
        wv_h.append(np.ascontiguousarray(Wv[kvrows].T).astype(BF))
        wo_h.append(np.ascontiguousarray(Wo[:, qrows].T).astype(BF))

    in_maps = []
    for core in range(8):
        b, hh = core // 2, core % 2
        in_maps.append({
            "xT": xTb[b],
            "wqT": wq_h[hh], "wkT": wk_h[hh], "wvT": wv_h[hh], "woT": wo_h[hh],
            "cosk": cosT, "sink": sinT,
            "maskmul": mask,
            "ones_d": ones, "onesb_d": onesb,
        })
    return in_maps


def assemble_output(results):
    out = np.empty((B, T, C), dtype=np.float32)
    for b in range(B):
        acc = results[2 * b]["outT"].astype(np.float32)
        acc += results[2 * b + 1]["outT"].astype(np.float32)
        out[b] = acc.T
    return out


def kernel(x, Wq, Wk, Wv, Wo, offset):
    nc = get_nc()
    in_maps = make_in_maps(x, Wq, Wk, Wv, Wo, offset)
    res = run_bass_kernel_spmd(nc, in_maps, core_ids=list(range(8)))
    return assemble_output(res.results)


# revision 7
# speedup vs baseline: 1.0403x; 1.0403x over previous
"""Trainium2 Bass kernel v3 for nn_MultiHeadAttention (GQA + RoPE + causal).

Same sharding/layout as v2 (4 batches x 2 head-halves, bf16 operands,
SBUF-resident x/Q/K/V/Y, multiplicative causal mask, DVE-tree softmax
denominator). v3 schedule/engine-balance changes:
  - x DMA split into 4 strips so K-proj starts after the first strip lands
  - wq resident (loaded once, early) instead of per-head strips
  - rope elementwise math in bf16 on DVE (2x/4x modes); trig tables bf16
  - last diagonal pair of each (head, block): S/exp computed only on the
    visible query region, masked region of P zeroed by Pool memset
  - pbs (softmax broadcast) copy moved to DVE; O-proj psum->sbuf copies
    alternate ACT/DVE to kill the per-oc stall
"""

import sys
import math

sys.path.insert(0, "/opt/trn_rl_repo")

import numpy as np
import ml_dtypes

import concourse.bacc as bacc
import concourse.mybir as mybir
import concourse.tile as tile
from concourse.bass_utils import run_bass_kernel_spmd

F32 = mybir.dt.float32
F32R = mybir.dt.float32r
BF16 = mybir.dt.bfloat16
AF = mybir.ActivationFunctionType

B, T, C = 4, 2048, 2048
NH, NKV, HD = 16, 4, 128
NHL = NH // 2
NKVL = NKV // 2
NREP = NH // NKV
ROPE_BASE = 10000.0
NCC = C // 128
NJ = T // 512
NTC = T // 128
BF = ml_dtypes.bfloat16


PHASES = "all"
POOL_MODE = 2


def _build_nc(nrep=1, extra_dma=0):
    nc = bacc.Bacc(trn_type="TRN2", name="mha_gqa_rope_v3")

    xT = nc.dram_tensor("xT", [C, T], BF16, kind="ExternalInput")
    wqT = nc.dram_tensor("wqT", [C, NHL * HD], BF16, kind="ExternalInput")
    wkT = nc.dram_tensor("wkT", [C, NKVL * HD], BF16, kind="ExternalInput")
    wvT = nc.dram_tensor("wvT", [C, NKVL * HD], BF16, kind="ExternalInput")
    woT = nc.dram_tensor("woT", [NHL * HD, C], BF16, kind="ExternalInput")
    cosk = nc.dram_tensor("cosk", [HD, T], BF16, kind="ExternalInput")
    sink = nc.dram_tensor("sink", [HD, T], BF16, kind="ExternalInput")
    maskmul = nc.dram_tensor("maskmul", [128, 4 * 512], BF16, kind="ExternalInput")
    ones_d = nc.dram_tensor("ones_d", [128, 128], F32R, kind="ExternalInput")
    onesb_d = nc.dram_tensor("onesb_d", [128, 1], BF16, kind="ExternalInput")
    outT = nc.dram_tensor("outT", [C, T], BF16, kind="ExternalOutput")

    with tile.TileContext(nc) as tc:
        with tc.tile_pool(name="const", bufs=1) as constp:
            ones_s = constp.tile([128, 128], F32R)
            nc.sync.dma_start(out=ones_s[:], in_=ones_d.ap())
            onesb_s = constp.tile([128, 1], BF16)
            nc.sync.dma_start(out=onesb_s[:], in_=onesb_d.ap())
            mask_s = constp.tile([128, 4, 512], BF16)
            nc.sync.dma_start(
                out=mask_s[:], in_=maskmul.ap().rearrange("p (i q) -> p i q", i=4)
            )

            for _rep in range(nrep):
                with tc.tile_pool(name="kvres", bufs=1) as kvres, \
                     tc.tile_pool(name="qy", bufs=1) as qyp:
                    kT_s = kvres.tile([128, NKVL, T], BF16)
                    v_s = kvres.tile([128, NTC, NKVL * HD], BF16)
                    q_s = qyp.tile([128, NHL, T], BF16)
                    y_s = qyp.tile([128, NHL, T], BF16)

                    with tc.tile_pool(name="xtrig", bufs=1) as xtp:
                        # small weights first so K-proj can start immediately
                        wk_s = xtp.tile([128, NCC, NKVL * HD], BF16, tag="wk")
                        nc.sync.dma_start(
                            out=wk_s[:, 0:8, :],
                            in_=wkT.ap()[0:1024, :].rearrange(
                                "(c p) k -> p c k", p=128),
                        )
                        x_s = xtp.tile([128, NCC, T], BF16, tag="x")

                        def _load_x_strip(c0, c1):
                            nc.sync.dma_start(
                                out=x_s[:, c0:c1, :],
                                in_=xT.ap()[c0 * 128:c1 * 128, :].rearrange(
                                    "(c p) t -> p c t", p=128
                                ),
                            )

                        _load_x_strip(0, 4)
                        nc.sync.dma_start(
                            out=wk_s[:, 8:16, :],
                            in_=wkT.ap()[1024:2048, :].rearrange(
                                "(c p) k -> p c k", p=128),
                        )
                        wv_s = xtp.tile([128, NCC, NKVL * HD], BF16, tag="wv")
                        nc.sync.dma_start(
                            out=wv_s[:],
                            in_=wvT.ap().rearrange("(c p) k -> p c k", p=128),
                        )
                        for s in range(1, 4):
                            _load_x_strip(4 * s, 4 * (s + 1))
                        cos_s = xtp.tile([HD, T], BF16, tag="cos")
                        nc.sync.dma_start(out=cos_s[:], in_=cosk.ap())
                        sin_s = xtp.tile([HD, T], BF16, tag="sin")
                        nc.sync.dma_start(out=sin_s[:], in_=sink.ap())
                        # wq strips streamed with prefetch (2 heads ahead)
                        wq_strips = []

                        def _issue_wq(wqp, h):
                            t = wqp.tile([128, NCC, 128], BF16, tag="wq")
                            nc.sync.dma_start(
                                out=t[:],
                                in_=wqT.ap()[:, h * 128:(h + 1) * 128].rearrange(
                                    "(c p) m -> p c m", p=128
                                ),
                            )
                            wq_strips.append(t)

                        # ---------------- KV stage ----------------
                        with tc.tile_pool(name="wqp", bufs=3) as wqp:
                          with tc.tile_pool(name="krope", bufs=3) as krp, \
                             tc.tile_pool(name="kps", bufs=2, space="PSUM") as kps, \
                             tc.tile_pool(name="vps", bufs=2, space="PSUM") as vps:
                            _issue_wq(wqp, 0)
                            _issue_wq(wqp, 1)
                            for g in range(NKVL):
                                psks = []
                                for half in range(2):
                                    psk_t = kps.tile([128, 1024], F32, tag="psk")
                                    psks.append(psk_t)
                                for cc in range(NCC):
                                    for half in range(2):
                                        for rb in range(2):
                                            nc.tensor.matmul(
                                                psks[half][:,
                                                           rb * 512:(rb + 1) * 512],
                                                wk_s[:, cc, g * 128:(g + 1) * 128],
                                                x_s[:, cc,
                                                    half * 1024 + rb * 512:
                                                    half * 1024 + (rb + 1) * 512],
                                                start=(cc == 0),
                                                stop=(cc == NCC - 1),
                                            )
                                for half in range(2):
                                    hsl = slice(half * 1024, (half + 1) * 1024)
                                    psk = psks[half]
                                    k0 = krp.tile([128, 1024], BF16, tag="k0")
                                    nc.scalar.copy(k0[:], psk[:])
                                    rot = krp.tile([128, 1024], BF16, tag="rot")
                                    nc.scalar.copy(rot[0:64, :], k0[64:128, :])
                                    nc.scalar.copy(rot[64:128, :], k0[0:64, :])
                                    t1 = krp.tile([128, 1024], BF16, tag="t1")
                                    nc.vector.tensor_mul(t1[:], k0[:], cos_s[:, hsl])
                                    nc.vector.tensor_mul(rot[:], rot[:],
                                                         sin_s[:, hsl])
                                    nc.vector.tensor_add(kT_s[:, g, hsl],
                                                         t1[:], rot[:])
                            for tc_i in range(NTC):
                                psv = vps.tile([128, NKVL * HD], F32, tag="psv")
                                for cc in range(NCC):
                                    nc.tensor.matmul(
                                        psv[:],
                                        x_s[:, cc, tc_i * 128:(tc_i + 1) * 128],
                                        wv_s[:, cc, :],
                                        start=(cc == 0),
                                        stop=(cc == NCC - 1),
                                    )
                                nc.scalar.copy(v_s[:, tc_i, :], psv[:])

                          # ------------- Q stage -------------
                          with tc.tile_pool(name="qrope", bufs=3) as qrp, \
                               tc.tile_pool(name="qps", bufs=2, space="PSUM") as qps:
                              for h in range(NHL):
                                if h + 2 < NHL:
                                    _issue_wq(wqp, h + 2)
                                wq_strip = wq_strips[h]
                                for half in range(2):
                                    hsl = slice(half * 1024, (half + 1) * 1024)
                                    psq = qps.tile([128, 1024], F32, tag="psq")
                                    for cc in range(NCC):
                                        for rb in range(2):
                                            nc.tensor.matmul(
                                                psq[:, rb * 512:(rb + 1) * 512],
                                                wq_strip[:, cc, :],
                                                x_s[:, cc,
                                                    half * 1024 + rb * 512:
                                                    half * 1024 + (rb + 1) * 512],
                                                start=(cc == 0),
                                                stop=(cc == NCC - 1),
                                            )
                                    q0 = qrp.tile([128, 1024], BF16, tag="q0")
                                    nc.scalar.copy(q0[:], psq[:])
                                    rot = qrp.tile([128, 1024], BF16, tag="qrot")
                                    nc.scalar.copy(rot[0:64, :], q0[64:128, :])
                                    nc.scalar.copy(rot[64:128, :], q0[0:64, :])
                                    t1 = qrp.tile([128, 1024], BF16, tag="qt1")
                                    nc.vector.tensor_mul(t1[:], q0[:], cos_s[:, hsl])
                                    nc.vector.tensor_mul(rot[:], rot[:], sin_s[:, hsl])
                                    nc.vector.tensor_add(q_s[:, h, hsl], t1[:], rot[:])

                    if PHASES == "proj":
                        # keep outputs live: copy a slice of q_s/kT_s out
                        with tc.tile_pool(name="dbg", bufs=1) as dbgp:
                            db = dbgp.tile([128, 512], BF16)
                            nc.vector.tensor_add(db[:], q_s[:, 0, 0:512],
                                                 kT_s[:, 0, 0:512])
                            nc.sync.dma_start(out=outT.ap()[0:128, 0:512],
                                              in_=db[:])
                        continue
                    # ---------------- attention + O-proj ----------------
                    with tc.tile_pool(name="wo", bufs=1) as wop, \
                         tc.tile_pool(name="pt", bufs=6) as ptp, \
                         tc.tile_pool(name="tree", bufs=12) as trp, \
                         tc.tile_pool(name="small", bufs=4) as smallp, \
                         tc.tile_pool(name="ob", bufs=4) as obp, \
                         tc.tile_pool(name="sps", bufs=2, space="PSUM") as sps, \
                         tc.tile_pool(name="po", bufs=2, space="PSUM") as ops, \
                         tc.tile_pool(name="den", bufs=1, space="PSUM") as dps, \
                         tc.tile_pool(name="pb", bufs=1, space="PSUM") as bps:
                        wo_s = wop.tile([128, NHL, C], BF16)
                        nc.sync.dma_start(
                            out=wo_s[:],
                            in_=woT.ap().rearrange("(yc p) o -> p yc o", p=128),
                        )
                        if extra_dma:
                            with tc.tile_pool(name="xd", bufs=2) as xdp:
                                for e in range(extra_dma * 4):
                                    xd = xdp.tile([128, 4, T], BF16, tag="xd")
                                    nc.sync.dma_start(
                                        out=xd[:],
                                        in_=xT.ap()[(e % 4) * 512:(e % 4 + 1) * 512, :]
                                        .rearrange("(c p) t -> p c t", p=128),
                                    )
                        self_cnt = [0]

                        def attend(j, h):
                                qsl = slice(j * 512, (j + 1) * 512)
                                g = h // NREP
                                po = ops.tile([128, 512], F32, tag="po")
                                npairs = 2 * j + 2
                                tree = []
                                for pp in range(npairs):
                                    last = pp == npairs - 1
                                    pss = sps.tile([128, 2, 512], F32, tag="pss")
                                    pt = ptp.tile([128, 2, 512], BF16, tag="pt")
                                    if not last:
                                        for ci in range(2):
                                            cc = 2 * pp + ci
                                            nc.tensor.matmul(
                                                pss[:, ci, :],
                                                kT_s[:, g, cc * 128:(cc + 1) * 128],
                                                q_s[:, h, qsl],
                                                start=True,
                                                stop=True,
                                            )
                                        nc.scalar.activation(
                                            pt[:].rearrange("p a q -> p (a q)"),
                                            pss[:].rearrange("p a q -> p (a q)"),
                                            AF.Exp,
                                        )
                                        for ci in range(2):
                                            cc = 2 * pp + ci
                                            if cc == 4 * j:
                                                nc.vector.tensor_mul(
                                                    pt[:, ci, 0:128],
                                                    pt[:, ci, 0:128],
                                                    mask_s[:, 0, 0:128],
                                                )
                                            elif cc == 4 * j + 1:
                                                eng1 = (nc.gpsimd if POOL_MODE == 2
                                                        else nc.vector)
                                                eng1.tensor_mul(
                                                    pt[:, ci, 0:256],
                                                    pt[:, ci, 0:256],
                                                    mask_s[:, 1, 0:256],
                                                )
                                    else:
                                        # diagonal pair: chunks 4j+2 (vis q>=256)
                                        # and 4j+3 (vis q>=384)
                                        for ci, vis in ((0, 256), (1, 384)):
                                            cc = 2 * pp + ci
                                            nc.tensor.matmul(
                                                pss[:, ci, vis:512],
                                                kT_s[:, g, cc * 128:(cc + 1) * 128],
                                                q_s[:, h,
                                                    j * 512 + vis:(j + 1) * 512],
                                                start=True,
                                                stop=True,
                                            )
                                            if POOL_MODE >= 1:
                                                nc.gpsimd.memset(pt[:, ci, 0:vis], 0.0)
                                            else:
                                                nc.vector.memset(pt[:, ci, 0:vis], 0.0)
                                            nc.scalar.activation(
                                                pt[:, ci, vis:512],
                                                pss[:, ci, vis:512],
                                                AF.Exp,
                                            )
                                            if ci == 0:
                                                nc.vector.tensor_mul(
                                                    pt[:, ci, 256:384],
                                                    pt[:, ci, 256:384],
                                                    mask_s[:, 2, 256:384],
                                                )
                                            else:
                                                eng3 = (nc.gpsimd if POOL_MODE == 2
                                                        else nc.vector)
                                                eng3.tensor_mul(
                                                    pt[:, ci, 384:512],
                                                    pt[:, ci, 384:512],
                                                    mask_s[:, 3, 384:512],
                                                )
                                    for ci in range(2):
                                        cc = 2 * pp + ci
                                        nc.tensor.matmul(
                                            po[:],
                                            v_s[:, cc, g * 128:(g + 1) * 128],
                                            pt[:, ci, :],
                                            start=(cc == 0),
                                            stop=(cc == 4 * j + 3),
                                        )
                                    def tree_add(out_, a_, b_):
                                        eng = nc.vector
                                        self_cnt[0] += 1
                                        eng.tensor_add(out_, a_, b_)

                                    node = trp.tile([128, 512], BF16, tag="tn")
                                    tree_add(node[:], pt[:, 0, :], pt[:, 1, :])
                                    lvl = 0
                                    while tree and tree[-1][0] == lvl:
                                        prev = tree.pop()[1]
                                        nxt = trp.tile([128, 512], BF16, tag="tn")
                                        tree_add(nxt[:], prev[:], node[:])
                                        node = nxt
                                        lvl += 1
                                    tree.append((lvl, node))
                                while len(tree) > 1:
                                    a = tree.pop()[1]
                                    b_ = tree.pop()[1]
                                    nxt = trp.tile([128, 512], BF16, tag="tn")
                                    nc.vector.tensor_add(nxt[:], a[:], b_[:])
                                    tree.append((99, nxt))
                                root = tree[0][1]
                                den = dps.tile([1, 512], F32, tag="den")
                                nc.tensor.matmul(
                                    den[:], onesb_s[:, 0:1], root[:],
                                    start=True, stop=True,
                                )
                                rec = smallp.tile([1, 512], F32R, tag="rec")
                                with nc.allow_low_precision(reason="softmax recip"):
                                    nc.vector.reciprocal(rec[:], den[:])
                                pb = bps.tile([128, 512], F32, tag="pb")
                                nc.tensor.matmul(
                                    pb[:], ones_s[0:1, :], rec[:],
                                    start=True, stop=True,
                                )
                                pbs = smallp.tile([128, 512], F32, tag="pbs")
                                if h % 2 == 0:
                                    nc.scalar.copy(pbs[:], pb[:])
                                else:
                                    nc.vector.tensor_copy(pbs[:], pb[:])
                                nc.vector.tensor_mul(
                                    y_s[:, h, qsl], po[:], pbs[:]
                                )
                        def oproj(j):
                            qsl = slice(j * 512, (j + 1) * 512)
                            for oc in range(NCC):
                                pso = ops.tile([128, 512], F32, tag="po")
                                for yc in range(NHL):
                                    nc.tensor.matmul(
                                        pso[:],
                                        wo_s[:, yc, oc * 128:(oc + 1) * 128],
                                        y_s[:, yc, qsl],
                                        start=(yc == 0),
                                        stop=(yc == NHL - 1),
                                    )
                                ob = obp.tile([128, 512], BF16, tag="ob")
                                if oc % 2 == 0:
                                    nc.scalar.copy(ob[:], pso[:])
                                else:
                                    nc.vector.tensor_copy(ob[:], pso[:])
                                nc.sync.dma_start(
                                    out=outT.ap()[oc * 128:(oc + 1) * 128, qsl],
                                    in_=ob[:],
                                )

                        for ja, jb in ((0, 1), (2, 3)):
                            for h in range(NHL):
                                attend(ja, h)
                                attend(jb, h)
                            oproj(ja)
                            oproj(jb)

    nc.finalize()
    return nc


_NC_CACHE = None


def get_nc():
    global _NC_CACHE
    if _NC_CACHE is None:
        _NC_CACHE = _build_nc()
    return _NC_CACHE


def build_nrep(nrep):
    return _build_nc(nrep=nrep, extra_dma=EXTRA_DMA)


EXTRA_DMA = 0


def _trig_tables(offset):
    inv_freq = 1.0 / (ROPE_BASE ** (np.arange(0, HD, 2, dtype=np.float64) / HD))
    pos = np.arange(offset, offset + T, dtype=np.float64)
    ang = pos[:, None] * inv_freq[None, :]
    cos = np.cos(ang)
    sin = np.sin(ang)
    cosT = np.concatenate([cos, cos], axis=1).T.astype(np.float32)
    sinT = np.concatenate([-sin, sin], axis=1).T.astype(np.float32)
    return (np.ascontiguousarray(cosT).astype(BF),
            np.ascontiguousarray(sinT).astype(BF))


def _mask_table():
    k = np.arange(128)[:, None]
    q = np.arange(512)[None, :]
    blocks = [(128 * i + k <= q).astype(np.float32) for i in range(4)]
    return np.concatenate(blocks, axis=1).astype(BF)


def make_in_maps(x, Wq, Wk, Wv, Wo, offset):
    x = np.asarray(x, dtype=np.float32)
    Wq = np.asarray(Wq, dtype=np.float32)
    Wk = np.asarray(Wk, dtype=np.float32)
    Wv = np.asarray(Wv, dtype=np.float32)
    Wo = np.asarray(Wo, dtype=np.float32)
    offset = int(np.asarray(offset))

    scale = 1.0 / math.sqrt(HD)
    cosT, sinT = _trig_tables(offset)
    mask = _mask_table()
    ones = np.ones((128, 128), dtype=np.float32)
    onesb = np.ones((128, 1), dtype=np.float32).astype(BF)

    xTb = [np.ascontiguousarray(x[b].T).astype(BF) for b in range(B)]
    wq_h, wk_h, wv_h, wo_h = [], [], [], []
    for hh in range(2):
        qrows = slice(hh * NHL * HD, (hh + 1) * NHL * HD)
        kvrows = slice(hh * NKVL * HD, (hh + 1) * NKVL * HD)
        wq_h.append(np.ascontiguousarray((Wq[qrows] * scale).T).astype(BF))
        wk_h.append(np.ascontiguousarray(Wk[kvrows].T).astype(BF))
        wv_h.append(np.ascontiguousarray(Wv[kvrows].T).astype(BF))
        wo_h.append(np.ascontiguousarray(Wo[:, qrows].T).astype(BF))

    in_maps = []
    for core in range(8):
        b, hh = core // 2, core % 2
        in_maps.append({
            "xT": xTb[b],
            "wqT": wq_h[hh], "wkT": wk_h[hh], "wvT": wv_h[hh], "woT": wo_h[hh],
            "cosk": cosT, "sink": sinT,
            "maskmul": mask,
            "ones_d": ones, "onesb_d": onesb,
        })
    return in_maps


def assemble_output(results):
    out = np.empty((B, T, C), dtype=np.float32)
    for b in range(B):
        acc = results[2 * b]["outT"].astype(np.float32)
        acc += results[2 * b + 1]["outT"].astype(np.float32)
        out[b] = acc.T
    return out


def kernel(x, Wq, Wk, Wv, Wo, offset):
    nc = get_nc()
    in_maps = make_in_maps(x, Wq, Wk, Wv, Wo, offset)
    res = run_bass_kernel_spmd(nc, in_maps, core_ids=list(range(8)))
    return assemble_output(res.results)
